# revision 59
# baseline (speedup 1.0000x reference)
"""ChannelGuidedAttn Trainium2 kernel.

Reference computation (per batch b):
    q  = x_pre[b]      reshaped (C, N),  C=512, N=H*W=4096
    kv = x_training[b] reshaped (C, N)
    energy[c,d] = <q[c,:], kv[d,:]>                      (C x C)
    att = softmax(max_d(energy) - energy, axis=-1)       == softmax(-energy)
        = exp(min_d(energy) - energy) / sum
    out = att @ kv  -> (C, H, W);  final softmax over W

Sharding: data-parallel over batch B=16 across 8 cores (2 batches/core).

Design: all transposes/casts are done on the HOST during sharding; the
device receives three pre-laid-out fp16 tensors per batch:
  - ktT[p, j, d] = kv[d, j*128+p]   (gemm1 moving operand, n on partitions)
  - qT [ct, p, j, c] = q[ct*128+c, j*128+p]  (gemm1 stationary, per c-tile)
  - kn [p, dt, n] = kv[dt*128+p, n] (gemm2 moving operand, natural layout)
Each layout is partition-major so every load DMA moves 8KB-contiguous
descriptor payloads (full bus efficiency, no 2x small-element penalty).

Device program per (batch, c-tile) task:
  g1(ct):  energy = qT(ct)^T @ ktT  (32 fp16 matmuls into one PSUM bank)
           min-reduce (DVE), att16 = exp(min - E) with sum accum (ACT),
           normalize att16 (DVE), attT via small DMA xbar transpose.
  g2(ct):  out = attT^T @ kn  (8 x 4 fp16 matmuls), exp (ACT, fp16 out),
           per-W-segment sums (DVE), reciprocal (DVE), normalize
           (alternating DVE/Pool), fp16 store per half-c-tile.

The 16 PE task slots and the single serialized DMA queue are explicitly
co-scheduled (see the schedule section): batch 0 runs its four gemm1s
back-to-back while loads stream in, batch 1's gemm1s interleave into batch
0's gemm2 phase, every load is positioned to complete just before its
consuming PE slot, and attT transposes (whose sem waits would head-of-line
block the SP DMA queue) are placed only where they are already ready.
PE runs at its arithmetic roofline with no mid-kernel stalls: total
~124us/core vs 109us of pure matmul (baseline was 308us).

gemm1 runs in plain fp16 (host-rounded inputs, fp32 PSUM accumulation):
measured absmax rel err 1.19e-2 against the f64 reference (gate 2e-2),
dominated by the fp16 rounding of q/kv feeding the huge (std ~64) energy
dot products. G1_MODE="f16q" adds a q-residual pass (err 7.1e-3) at
+27us PE if a larger margin is ever needed.
"""

import sys

import numpy as np

for _p in ("/opt/trn_rl_repo", "/root/.axon_site/_ro/trn_rl_repo"):
    if _p not in sys.path:
        sys.path.append(_p)

B = 16
N_CORES = 8
B_PER_CORE = B // N_CORES
C = 512
H = 64
W = 64
N = H * W
CT = C // 128  # 4 c-tiles / d-tiles
NJ = N // 128  # 32 n-chunks of 128

G1_MODE = "f16"  # "f16" (plain fp16) | "f16q" (q hi/lo split, 2-pass)


def build_program(g1_mode=None):
    from contextlib import ExitStack

    import concourse.mybir as mybir
    import concourse.tile as tile
    from concourse import bacc

    if g1_mode is None:
        g1_mode = G1_MODE
    assert g1_mode in ("f16", "f16q")
    q_split = g1_mode == "f16q"

    f32 = mybir.dt.float32
    f16 = mybir.dt.float16
    Alu = mybir.AluOpType
    Act = mybir.ActivationFunctionType
    Axis = mybir.AxisListType

    nc = bacc.Bacc()
    # Host-prepared layouts (see module docstring).
    ktT = nc.declare_dram_parameter("ktT", [B_PER_CORE, 128, NJ, C], f16, isOutput=False)
    qT = nc.declare_dram_parameter("qT", [B_PER_CORE, CT, 128, NJ, 128], f16, isOutput=False)
    if q_split:
        qlT = nc.declare_dram_parameter(
            "qlT", [B_PER_CORE, CT, 128, NJ, 128], f16, isOutput=False
        )
    kn = nc.declare_dram_parameter("kn", [B_PER_CORE, 128, CT, N], f16, isOutput=False)
    out = nc.declare_dram_parameter("out", [B_PER_CORE, C, N], f16, isOutput=True)

    with tile.TileContext(nc) as tc, ExitStack() as ctx:
        ktp = ctx.enter_context(tc.tile_pool(name="ktp", bufs=2))
        knp = ctx.enter_context(tc.tile_pool(name="knp", bufs=2))
        qtp = ctx.enter_context(tc.tile_pool(name="qtp", bufs=5 + 3 * q_split))
        attp = ctx.enter_context(tc.tile_pool(name="attp", bufs=2))
        ostp = ctx.enter_context(tc.tile_pool(name="ostp", bufs=5))
        small = ctx.enter_context(tc.tile_pool(name="small", bufs=4))
        ps_e = ctx.enter_context(tc.tile_pool(name="ps_e", bufs=3, space="PSUM"))
        ps_o = ctx.enter_context(tc.tile_pool(name="ps_o", bufs=4, space="PSUM"))

        # Per-batch SBUF tiles, created/rotated on demand.
        kt_sb = {}
        kn_sb = {}
        qt_sb = {}
        ql_sb = {}
        att_16 = {}
        att_T = {}

        def emit_ktT_chunks(b, chunks, nch=8):
            # default 8 chunks of 4 j's each (~1.6us apiece on the DMA queue)
            if b not in kt_sb:
                kt_sb[b] = ktp.tile([128, NJ, C], f16, tag="ktT", name=f"ktT_{b}")
            for g in chunks:
                js = slice(g * (NJ // nch), (g + 1) * (NJ // nch))
                nc.sync.dma_start(out=kt_sb[b][:, js, :], in_=ktT[b, :, js, :])

        def emit_kn_chunks(b, chunks):
            if b not in kn_sb:
                kn_sb[b] = knp.tile([128, CT, N], f16, tag="kn", name=f"kn_{b}")
            for dt in chunks:
                nc.sync.dma_start(out=kn_sb[b][:, dt, :], in_=kn[b, :, dt, :])

        def emit_qT_load(b, ct, halves=1):
            qt_sb[(b, ct)] = qtp.tile([128, NJ, 128], f16, tag="qT", name=f"qT_{b}_{ct}")
            for hh in range(halves):
                js = slice(hh * (NJ // halves), (hh + 1) * (NJ // halves))
                nc.sync.dma_start(out=qt_sb[(b, ct)][:, js, :], in_=qT[b, ct, :, js, :])
            if q_split:
                ql_sb[(b, ct)] = qtp.tile(
                    [128, NJ, 128], f16, tag="qlT", name=f"qlT_{b}_{ct}"
                )
                nc.sync.dma_start(out=ql_sb[(b, ct)], in_=qlT[b, ct])

        def emit_g1(b, ct):
            # energy for this c-tile, then attention row softmax + transpose
            e_ps = ps_e.tile([128, C], f32, tag="ps_e")
            qt = qt_sb.pop((b, ct))
            ql = ql_sb.pop((b, ct), None)
            kt = kt_sb[b]
            for j in range(NJ):
                last = j == NJ - 1
                nc.tensor.matmul(
                    e_ps, qt[:, j, :], kt[:, j, :],
                    start=(j == 0), stop=(last and not q_split),
                )
                if q_split:
                    nc.tensor.matmul(
                        e_ps, ql[:, j, :], kt[:, j, :], start=False, stop=last
                    )
            min_t = small.tile([128, 1], f32, tag="min")
            nc.vector.tensor_reduce(min_t, e_ps, axis=Axis.X, op=Alu.min)
            att16 = attp.tile([128, C], f16, tag="att16")
            den = small.tile([128, 1], f32, tag="den")
            nc.scalar.activation(
                out=att16, in_=e_ps, func=Act.Exp, bias=min_t, scale=-1.0,
                accum_out=den,
            )
            rden = small.tile([128, 1], f32, tag="rden")
            nc.vector.reciprocal(rden, den)
            nc.vector.tensor_scalar_mul(att16, att16, rden)
            att_16[(b, ct)] = att16

        def emit_attT(b, ct):
            # placed explicitly in the DMA queue: late enough that its sem
            # wait (att16 ready) never head-of-line-blocks loads behind it
            att_T[(b, ct)] = attp.tile(
                [128, CT, 128], f16, tag="attT", name=f"attT_{b}_{ct}"
            )
            nc.sync.dma_start_transpose(att_T[(b, ct)], att_16.pop((b, ct)))

        def emit_g2(b, ct, tail=False, last=False):
            # out rows for this c-tile: gemm2 + final softmax over W + store.
            # tail: the final two tasks — DVE is the scarce engine there, so
            # push most normalizes to Pool (DVE keeps nj 5,7 for low latency).
            attT = att_T.pop((b, ct))
            knb = kn_sb[b]
            for h in range(2):
                o16 = ostp.tile([128, 2048], f16, tag="ost")  # 4 nj chunks of 512

                def post(o_ps, k, nj, lo, hi, seg_tag, store):
                    # softmax over W + optional store for columns [lo, hi)
                    sl = slice(k * 512 + lo, k * 512 + hi)
                    o16v = o16[:, sl].rearrange("p (s w) -> p s w", w=W)
                    nc.scalar.activation(
                        out=o16v,
                        in_=o_ps[:, : hi - lo].rearrange("p (s w) -> p s w", w=W),
                        func=Act.Exp,
                    )
                    nseg = (hi - lo) // W
                    ssum = small.tile([128, nseg], f32, tag="ssum" + seg_tag)
                    nc.vector.tensor_reduce(ssum, o16v, axis=Axis.X, op=Alu.add)
                    rsum = small.tile([128, nseg], f32, tag="rsum" + seg_tag)
                    nc.vector.reciprocal(rsum, ssum)
                    if tail:
                        on_pool = nj != 7
                    else:
                        on_pool = nj % 2 == 0
                    eng = nc.gpsimd if on_pool else nc.vector
                    eng.tensor_tensor(
                        out=o16v,
                        in0=o16v,
                        in1=rsum[:, :, None].to_broadcast(o16v.shape),
                        op=Alu.mult,
                    )
                    if store:
                        nc.sync.dma_start(
                            out=out[
                                b,
                                ct * 128 : (ct + 1) * 128,
                                nj * 512 + lo : nj * 512 + hi,
                            ],
                            in_=o16[:, sl],
                        )

                def mm(nj, lo, hi, o_ps):
                    for dt in range(CT):
                        nc.tensor.matmul(
                            o_ps[:, : hi - lo],
                            attT[:, dt, :],
                            knb[:, dt, nj * 512 + lo : nj * 512 + hi],
                            start=(dt == 0),
                            stop=(dt == CT - 1),
                        )

                for k in range(4):
                    nj = h * 4 + k
                    o_ps = ps_o.tile([128, 512], f32, tag="ps_o")
                    mm(nj, 0, 512, o_ps)
                    post(o_ps, k, nj, 0, 512, "", store=last)
                if not last:
                    nc.sync.dma_start(
                        out=out[
                            b, ct * 128 : (ct + 1) * 128, h * 2048 : (h + 1) * 2048
                        ],
                        in_=o16,
                    )

        # ---- explicit software-pipelined schedule (2 batches) ----
        # DMA queue and the in-order PE stream are co-scheduled: batch 0 runs
        # all four gemm1s first (its kn/attT can't be ready earlier anyway);
        # batch 1 prefetches are slotted so no PE task ever waits on a load.
        assert B_PER_CORE == 2
        # Hand-scheduled against the cost model: PE slot sequence is
        # g1(00..03), g2(00), g2(01), g1(10), g2(02), g1(11), g2(03),
        # g1(12), g2(10), g1(13), g2(11), g2(12), g2(13); every DMA is
        # placed so it completes just before its consuming PE slot and no
        # sem-waiting DMA ever blocks a load queued behind it.
        emit_qT_load(0, 0, halves=2)
        emit_ktT_chunks(0, range(0, 8), nch=16)
        emit_qT_load(0, 1, halves=2)
        emit_ktT_chunks(0, range(4, 8), nch=8)
        emit_qT_load(0, 2)
        emit_qT_load(0, 3)
        emit_kn_chunks(0, [0, 1, 2])
        emit_g1(0, 0)
        emit_attT(0, 0)
        emit_kn_chunks(0, [3])
        emit_g1(0, 1)
        emit_attT(0, 1)
        emit_g1(0, 2)
        emit_attT(0, 2)
        emit_qT_load(1, 0)
        emit_ktT_chunks(1, range(0, 2))
        emit_g1(0, 3)
        emit_attT(0, 3)
        emit_ktT_chunks(1, range(2, 8))
        emit_g2(0, 0)  # st00 x2
        emit_qT_load(1, 1)
        emit_kn_chunks(1, [0, 1])
        emit_g2(0, 1)  # st01 x2
        emit_g1(1, 0)
        emit_qT_load(1, 2)
        emit_kn_chunks(1, [2, 3])
        emit_attT(1, 0)
        emit_g2(0, 2)  # st02 x2
        emit_g1(1, 1)
        emit_qT_load(1, 3)
        emit_attT(1, 1)
        emit_g2(0, 3)  # st03 x2
        emit_g1(1, 2)
        emit_attT(1, 2)
        emit_g2(1, 0)  # st10 x2
        emit_g1(1, 3)
        emit_attT(1, 3)
        emit_g2(1, 1)  # st11 x2
        emit_g2(1, 2, tail=True)
        emit_g2(1, 3, tail=True, last=True)

    nc.finalize()
    return nc


def prepare_in_maps(x_training, x_pre, g1_mode=None):
    """Host-side shard + layout prep. Returns per-core input dicts."""
    if g1_mode is None:
        g1_mode = G1_MODE
    xt = np.asarray(x_training, dtype=np.float32).reshape(B, C, N)
    xp = np.asarray(x_pre, dtype=np.float32).reshape(B, C, N)
    xt16 = xt.astype(np.float16)
    xp16 = xp.astype(np.float16)

    # ktT[b, p, j, d] = kv[b, d, j*128+p]
    ktT = np.ascontiguousarray(
        xt16.reshape(B, C, NJ, 128).transpose(0, 3, 2, 1)
    )
    # qT[b, ct, p, j, c] = q[b, ct*128+c, j*128+p]
    qT = np.ascontiguousarray(
        xp16.reshape(B, CT, 128, NJ, 128).transpose(0, 1, 4, 3, 2)
    )
    # kn[b, p, dt, n] = kv[b, dt*128+p, n]
    knat = np.ascontiguousarray(xt16.reshape(B, CT, 128, N).transpose(0, 2, 1, 3))
    if g1_mode == "f16q":
        ql = (xp - xp16.astype(np.float32)).astype(np.float16)
        qlT = np.ascontiguousarray(
            ql.reshape(B, CT, 128, NJ, 128).transpose(0, 1, 4, 3, 2)
        )

    in_maps = []
    for i in range(N_CORES):
        sl = slice(i * B_PER_CORE, (i + 1) * B_PER_CORE)
        m = {"ktT": ktT[sl], "qT": qT[sl], "kn": knat[sl]}
        if g1_mode == "f16q":
            m["qlT"] = qlT[sl]
        in_maps.append(m)
    return in_maps


def kernel(x_training: np.ndarray, x_pre: np.ndarray) -> np.ndarray:
    from concourse.bass_utils import run_bass_kernel_spmd

    nc = build_program()
    in_maps = prepare_in_maps(x_training, x_pre)
    res = run_bass_kernel_spmd(nc, in_maps, list(range(N_CORES)))
    outs = [np.asarray(r["out"]) for r in res.results]
    return np.concatenate(outs, axis=0).reshape(B, C, H, W).astype(np.float32)


# revision 60
# speedup vs baseline: 1.0022x; 1.0022x over previous
"""ChannelGuidedAttn Trainium2 kernel.

Reference computation (per batch b):
    q  = x_pre[b]      reshaped (C, N),  C=512, N=H*W=4096
    kv = x_training[b] reshaped (C, N)
    energy[c,d] = <q[c,:], kv[d,:]>                      (C x C)
    att = softmax(max_d(energy) - energy, axis=-1)       == softmax(-energy)
        = exp(min_d(energy) - energy) / sum
    out = att @ kv  -> (C, H, W);  final softmax over W

Sharding: data-parallel over batch B=16 across 8 cores (2 batches/core).

Design: all transposes/casts are done on the HOST during sharding; the
device receives three pre-laid-out fp16 tensors per batch:
  - ktT[p, j, d] = kv[d, j*128+p]   (gemm1 moving operand, n on partitions)
  - qT [ct, p, j, c] = q[ct*128+c, j*128+p]  (gemm1 stationary, per c-tile)
  - kn [p, dt, n] = kv[dt*128+p, n] (gemm2 moving operand, natural layout)
Each layout is partition-major so every load DMA moves 8KB-contiguous
descriptor payloads (full bus efficiency, no 2x small-element penalty).

Device program per (batch, c-tile) task:
  g1(ct):  energy = qT(ct)^T @ ktT  (32 fp16 matmuls into one PSUM bank)
           min-reduce (DVE), att16 = exp(min - E) with sum accum (ACT),
           normalize att16 (DVE), attT via small DMA xbar transpose.
  g2(ct):  out = attT^T @ kn  (8 x 4 fp16 matmuls), exp (ACT, fp16 out),
           per-W-segment sums (DVE), reciprocal (DVE), normalize
           (alternating DVE/Pool), fp16 store per half-c-tile.

The 16 PE task slots and the single serialized DMA queue are explicitly
co-scheduled (see the schedule section): batch 0 runs its four gemm1s
back-to-back while loads stream in, batch 1's gemm1s interleave into batch
0's gemm2 phase, every load is positioned to complete just before its
consuming PE slot, and attT transposes (whose sem waits would head-of-line
block the SP DMA queue) are placed only where they are already ready.
PE runs at its arithmetic roofline with no mid-kernel stalls: total
~125us/core vs 109us of pure matmul (baseline was 308us).

gemm1 runs in plain fp16 (host-rounded inputs, fp32 PSUM accumulation):
measured absmax rel err 1.19e-2 against the f64 reference (gate 2e-2),
dominated by the fp16 rounding of q/kv feeding the huge (std ~64) energy
dot products. G1_MODE="f16q" adds a q-residual pass (err 7.1e-3) at
+27us PE if a larger margin is ever needed.
"""

import sys

import numpy as np

for _p in ("/opt/trn_rl_repo", "/root/.axon_site/_ro/trn_rl_repo"):
    if _p not in sys.path:
        sys.path.append(_p)

B = 16
N_CORES = 8
B_PER_CORE = B // N_CORES
C = 512
H = 64
W = 64
N = H * W
CT = C // 128  # 4 c-tiles / d-tiles
NJ = N // 128  # 32 n-chunks of 128

G1_MODE = "f16"  # "f16" (plain fp16) | "f16q" (q hi/lo split, 2-pass)


def build_program(g1_mode=None):
    from contextlib import ExitStack

    import concourse.mybir as mybir
    import concourse.tile as tile
    from concourse import bacc

    if g1_mode is None:
        g1_mode = G1_MODE
    assert g1_mode in ("f16", "f16q")
    q_split = g1_mode == "f16q"

    f32 = mybir.dt.float32
    f16 = mybir.dt.float16
    Alu = mybir.AluOpType
    Act = mybir.ActivationFunctionType
    Axis = mybir.AxisListType

    nc = bacc.Bacc()
    # Host-prepared layouts (see module docstring).
    ktT = nc.declare_dram_parameter("ktT", [B_PER_CORE, 128, NJ, C], f16, isOutput=False)
    qT = nc.declare_dram_parameter("qT", [B_PER_CORE, CT, 128, NJ, 128], f16, isOutput=False)
    if q_split:
        qlT = nc.declare_dram_parameter(
            "qlT", [B_PER_CORE, CT, 128, NJ, 128], f16, isOutput=False
        )
    kn = nc.declare_dram_parameter("kn", [B_PER_CORE, 128, CT, N], f16, isOutput=False)
    out = nc.declare_dram_parameter("out", [B_PER_CORE, C, N], f16, isOutput=True)

    with tile.TileContext(nc) as tc, ExitStack() as ctx:
        ktp = ctx.enter_context(tc.tile_pool(name="ktp", bufs=2))
        knp = ctx.enter_context(tc.tile_pool(name="knp", bufs=2))
        qtp = ctx.enter_context(tc.tile_pool(name="qtp", bufs=5 + 3 * q_split))
        attp = ctx.enter_context(tc.tile_pool(name="attp", bufs=2))
        ostp = ctx.enter_context(tc.tile_pool(name="ostp", bufs=5))
        small = ctx.enter_context(tc.tile_pool(name="small", bufs=4))
        ps_e = ctx.enter_context(tc.tile_pool(name="ps_e", bufs=3, space="PSUM"))
        ps_o = ctx.enter_context(tc.tile_pool(name="ps_o", bufs=4, space="PSUM"))

        # Per-batch SBUF tiles, created/rotated on demand.
        kt_sb = {}
        kn_sb = {}
        qt_sb = {}
        ql_sb = {}
        att_16 = {}
        att_T = {}

        def emit_ktT_chunks(b, chunks, nch=8):
            # default 8 chunks of 4 j's each (~1.6us apiece on the DMA queue)
            if b not in kt_sb:
                kt_sb[b] = ktp.tile([128, NJ, C], f16, tag="ktT", name=f"ktT_{b}")
            for g in chunks:
                js = slice(g * (NJ // nch), (g + 1) * (NJ // nch))
                nc.sync.dma_start(out=kt_sb[b][:, js, :], in_=ktT[b, :, js, :])

        def emit_kn_chunks(b, chunks):
            if b not in kn_sb:
                kn_sb[b] = knp.tile([128, CT, N], f16, tag="kn", name=f"kn_{b}")
            for dt in chunks:
                nc.sync.dma_start(out=kn_sb[b][:, dt, :], in_=kn[b, :, dt, :])

        def emit_qT_load(b, ct, halves=1):
            qt_sb[(b, ct)] = qtp.tile([128, NJ, 128], f16, tag="qT", name=f"qT_{b}_{ct}")
            for hh in range(halves):
                js = slice(hh * (NJ // halves), (hh + 1) * (NJ // halves))
                nc.sync.dma_start(out=qt_sb[(b, ct)][:, js, :], in_=qT[b, ct, :, js, :])
            if q_split:
                ql_sb[(b, ct)] = qtp.tile(
                    [128, NJ, 128], f16, tag="qlT", name=f"qlT_{b}_{ct}"
                )
                nc.sync.dma_start(out=ql_sb[(b, ct)], in_=qlT[b, ct])

        def emit_g1(b, ct):
            # energy for this c-tile, then attention row softmax + transpose
            e_ps = ps_e.tile([128, C], f32, tag="ps_e")
            qt = qt_sb.pop((b, ct))
            ql = ql_sb.pop((b, ct), None)
            kt = kt_sb[b]
            for j in range(NJ):
                last = j == NJ - 1
                nc.tensor.matmul(
                    e_ps, qt[:, j, :], kt[:, j, :],
                    start=(j == 0), stop=(last and not q_split),
                )
                if q_split:
                    nc.tensor.matmul(
                        e_ps, ql[:, j, :], kt[:, j, :], start=False, stop=last
                    )
            min_t = small.tile([128, 1], f32, tag="min")
            nc.vector.tensor_reduce(min_t, e_ps, axis=Axis.X, op=Alu.min)
            att16 = attp.tile([128, C], f16, tag="att16")
            den = small.tile([128, 1], f32, tag="den")
            nc.scalar.activation(
                out=att16, in_=e_ps, func=Act.Exp, bias=min_t, scale=-1.0,
                accum_out=den,
            )
            rden = small.tile([128, 1], f32, tag="rden")
            nc.vector.reciprocal(rden, den)
            nc.vector.tensor_scalar_mul(att16, att16, rden)
            att_16[(b, ct)] = att16

        def emit_attT(b, ct):
            # placed explicitly in the DMA queue: late enough that its sem
            # wait (att16 ready) never head-of-line-blocks loads behind it
            att_T[(b, ct)] = attp.tile(
                [128, CT, 128], f16, tag="attT", name=f"attT_{b}_{ct}"
            )
            nc.sync.dma_start_transpose(att_T[(b, ct)], att_16.pop((b, ct)))

        def emit_g2(b, ct, tail=False, last=False):
            # out rows for this c-tile: gemm2 + final softmax over W + store.
            # tail: the final two tasks — DVE is the scarce engine there, so
            # push most normalizes to Pool (DVE keeps nj 5,7 for low latency).
            attT = att_T.pop((b, ct))
            knb = kn_sb[b]
            for h in range(2):
                o16 = ostp.tile([128, 2048], f16, tag="ost")  # 4 nj chunks of 512

                def post(o_ps, k, nj, lo, hi, seg_tag, store):
                    # softmax over W + optional store for columns [lo, hi)
                    sl = slice(k * 512 + lo, k * 512 + hi)
                    o16v = o16[:, sl].rearrange("p (s w) -> p s w", w=W)
                    nc.scalar.activation(
                        out=o16v,
                        in_=o_ps[:, : hi - lo].rearrange("p (s w) -> p s w", w=W),
                        func=Act.Exp,
                    )
                    nseg = (hi - lo) // W
                    ssum = small.tile([128, nseg], f32, tag="ssum" + seg_tag)
                    nc.vector.tensor_reduce(ssum, o16v, axis=Axis.X, op=Alu.add)
                    rsum = small.tile([128, nseg], f32, tag="rsum" + seg_tag)
                    nc.vector.reciprocal(rsum, ssum)
                    if last and nj == 7:
                        # final chunk: split the normalize across DVE and
                        # Pool in parallel to shorten the last store's gate
                        hs = nseg // 2
                        for i, eng in ((0, nc.vector), (1, nc.gpsimd)):
                            ov = o16v[:, i * hs : (i + 1) * hs, :]
                            eng.tensor_tensor(
                                out=ov,
                                in0=ov,
                                in1=rsum[:, i * hs : (i + 1) * hs, None]
                                .to_broadcast(ov.shape),
                                op=Alu.mult,
                            )
                    else:
                        if tail:
                            on_pool = nj != 7
                        else:
                            on_pool = nj % 2 == 0
                        eng = nc.gpsimd if on_pool else nc.vector
                        eng.tensor_tensor(
                            out=o16v,
                            in0=o16v,
                            in1=rsum[:, :, None].to_broadcast(o16v.shape),
                            op=Alu.mult,
                        )
                    if store:
                        nc.sync.dma_start(
                            out=out[
                                b,
                                ct * 128 : (ct + 1) * 128,
                                nj * 512 + lo : nj * 512 + hi,
                            ],
                            in_=o16[:, sl],
                        )

                def mm(nj, lo, hi, o_ps):
                    for dt in range(CT):
                        nc.tensor.matmul(
                            o_ps[:, : hi - lo],
                            attT[:, dt, :],
                            knb[:, dt, nj * 512 + lo : nj * 512 + hi],
                            start=(dt == 0),
                            stop=(dt == CT - 1),
                        )

                for k in range(4):
                    nj = h * 4 + k
                    o_ps = ps_o.tile([128, 512], f32, tag="ps_o")
                    mm(nj, 0, 512, o_ps)
                    post(o_ps, k, nj, 0, 512, "", store=last)
                if not last:
                    nc.sync.dma_start(
                        out=out[
                            b, ct * 128 : (ct + 1) * 128, h * 2048 : (h + 1) * 2048
                        ],
                        in_=o16,
                    )

        # ---- explicit software-pipelined schedule (2 batches) ----
        # DMA queue and the in-order PE stream are co-scheduled: batch 0 runs
        # all four gemm1s first (its kn/attT can't be ready earlier anyway);
        # batch 1 prefetches are slotted so no PE task ever waits on a load.
        assert B_PER_CORE == 2
        # Hand-scheduled against the cost model: PE slot sequence is
        # g1(00..03), g2(00), g2(01), g1(10), g2(02), g1(11), g2(03),
        # g1(12), g2(10), g1(13), g2(11), g2(12), g2(13); every DMA is
        # placed so it completes just before its consuming PE slot and no
        # sem-waiting DMA ever blocks a load queued behind it.
        emit_qT_load(0, 0, halves=2)
        emit_ktT_chunks(0, range(0, 8), nch=16)
        emit_qT_load(0, 1, halves=2)
        emit_ktT_chunks(0, range(4, 8), nch=8)
        emit_qT_load(0, 2)
        emit_qT_load(0, 3)
        emit_kn_chunks(0, [0, 1, 2])
        emit_g1(0, 0)
        emit_attT(0, 0)
        emit_kn_chunks(0, [3])
        emit_g1(0, 1)
        emit_attT(0, 1)
        emit_g1(0, 2)
        emit_attT(0, 2)
        emit_qT_load(1, 0)
        emit_ktT_chunks(1, range(0, 2))
        emit_g1(0, 3)
        emit_attT(0, 3)
        emit_ktT_chunks(1, range(2, 8))
        emit_g2(0, 0)  # st00 x2
        emit_qT_load(1, 1)
        emit_kn_chunks(1, [0, 1])
        emit_g2(0, 1)  # st01 x2
        emit_g1(1, 0)
        emit_qT_load(1, 2)
        emit_kn_chunks(1, [2, 3])
        emit_attT(1, 0)
        emit_g2(0, 2)  # st02 x2
        emit_g1(1, 1)
        emit_qT_load(1, 3)
        emit_attT(1, 1)
        emit_g2(0, 3)  # st03 x2
        emit_g1(1, 2)
        emit_attT(1, 2)
        emit_g2(1, 0)  # st10 x2
        emit_g1(1, 3)
        emit_attT(1, 3)
        emit_g2(1, 1)  # st11 x2
        emit_g2(1, 2, tail=True)
        emit_g2(1, 3, tail=True, last=True)

    nc.finalize()
    return nc


def prepare_in_maps(x_training, x_pre, g1_mode=None):
    """Host-side shard + layout prep. Returns per-core input dicts."""
    if g1_mode is None:
        g1_mode = G1_MODE
    xt = np.asarray(x_training, dtype=np.float32).reshape(B, C, N)
    xp = np.asarray(x_pre, dtype=np.float32).reshape(B, C, N)
    xt16 = xt.astype(np.float16)
    xp16 = xp.astype(np.float16)

    # ktT[b, p, j, d] = kv[b, d, j*128+p]
    ktT = np.ascontiguousarray(
        xt16.reshape(B, C, NJ, 128).transpose(0, 3, 2, 1)
    )
    # qT[b, ct, p, j, c] = q[b, ct*128+c, j*128+p]
    qT = np.ascontiguousarray(
        xp16.reshape(B, CT, 128, NJ, 128).transpose(0, 1, 4, 3, 2)
    )
    # kn[b, p, dt, n] = kv[b, dt*128+p, n]
    knat = np.ascontiguousarray(xt16.reshape(B, CT, 128, N).transpose(0, 2, 1, 3))
    if g1_mode == "f16q":
        ql = (xp - xp16.astype(np.float32)).astype(np.float16)
        qlT = np.ascontiguousarray(
            ql.reshape(B, CT, 128, NJ, 128).transpose(0, 1, 4, 3, 2)
        )

    in_maps = []
    for i in range(N_CORES):
        sl = slice(i * B_PER_CORE, (i + 1) * B_PER_CORE)
        m = {"ktT": ktT[sl], "qT": qT[sl], "kn": knat[sl]}
        if g1_mode == "f16q":
            m["qlT"] = qlT[sl]
        in_maps.append(m)
    return in_maps


def kernel(x_training: np.ndarray, x_pre: np.ndarray) -> np.ndarray:
    from concourse.bass_utils import run_bass_kernel_spmd

    nc = build_program()
    in_maps = prepare_in_maps(x_training, x_pre)
    res = run_bass_kernel_spmd(nc, in_maps, list(range(N_CORES)))
    outs = [np.asarray(r["out"]) for r in res.results]
    return np.concatenate(outs, axis=0).reshape(B, C, H, W).astype(np.float32)
